# revision 6
# baseline (speedup 1.0000x reference)
"""Trainium2 Bass kernel for nn_MessageFunction (GNN message passing).

Computes msg[b,o,n] = sum_d We[o,d]*e_vw[b,d,n] + sum_d Ww[o,d]*h_w[b,d,n]
                      + (be+bw)[o]
for B=128, D=768, N=256, data-parallel over B across 8 NeuronCores.

Strategy per core (16 batches):
- Host pre-transposes the weights (WeT = We.T, WwT = Ww.T) and fuses the
  biases; the device keeps both weight matrices resident in SBUF as
  float32r (TF32-like matmul dtype, full PE rate at moving dim >= 256,
  ~1.5e-4 rel err).
- Batches are processed in pairs so each matmul streams 512 moving
  columns (one full PSUM bank). For each pair and each of the 6 output
  tiles, 12 accumulating matmuls (6 k-tiles x {e,h}) produce a [128,512]
  PSUM tile; ScalarE adds the bias while copying PSUM->SBUF; one DMA
  stores the result.
"""
import numpy as np
import concourse.tile as tile
from concourse import bacc, mybir
from concourse.bass_utils import run_bass_kernel_spmd

B, D, NN = 128, 768, 256
NCORES = 8
BPC = B // NCORES          # 16 batches per core
PAIR = 2                   # batches fused per matmul moving block
NPAIR = BPC // PAIR        # 8
NCOL = PAIR * NN           # 512 moving columns
KT = D // 128              # 6 contraction tiles per input matrix
MT = D // 128              # 6 output row tiles
F32 = mybir.dt.float32
F32R = mybir.dt.float32r


def build(repeat: int = 1, loop_repeat: int = 1):
    nc = bacc.Bacc("TRN2", target_bir_lowering=False, debug=False,
                   num_devices=NCORES)
    e = nc.dram_tensor("e", [BPC, D, NN], F32, kind="ExternalInput").ap()
    h = nc.dram_tensor("h", [BPC, D, NN], F32, kind="ExternalInput").ap()
    weT = nc.dram_tensor("weT", [D, D], F32, kind="ExternalInput").ap()
    wwT = nc.dram_tensor("wwT", [D, D], F32, kind="ExternalInput").ap()
    bias = nc.dram_tensor("bias", [D], F32, kind="ExternalInput").ap()
    out = nc.dram_tensor("out", [BPC, D, NN], F32, kind="ExternalOutput").ap()

    e_v = e.rearrange("b (k p) n -> p k b n", p=128)        # [128,6,16,256]
    h_v = h.rearrange("b (k p) n -> p k b n", p=128)
    weT_v = weT.rearrange("(k p) (m q) -> p k m q", p=128, q=128)
    wwT_v = wwT.rearrange("(k p) (m q) -> p k m q", p=128, q=128)
    bias_v = bias.rearrange("(m p) -> p m", p=128)          # [128,6]
    out_v = out.rearrange("b (m p) n -> p m b n", p=128)    # [128,6,16,256]

    with tile.TileContext(nc) as tc:
        with (
            tc.tile_pool(name="wpool", bufs=1) as wpool,
            tc.tile_pool(name="xpool", bufs=3) as xpool,
            tc.tile_pool(name="opool", bufs=4) as opool,
            tc.tile_pool(name="pspool", bufs=4, space="PSUM") as pspool,
        ):
            # Weights resident for the whole kernel; split the load per
            # m-tile so the first matmul group only waits on its own slice.
            we_t = wpool.tile([128, KT, MT, 128], F32R)
            ww_t = wpool.tile([128, KT, MT, 128], F32R)
            bias_t = wpool.tile([128, MT], F32)
            nc.sync.dma_start(bias_t[:], bias_v)
            for m in range(MT):
                nc.sync.dma_start(we_t[:, :, m, :], weT_v[:, :, m, :].bitcast(F32R))
                nc.sync.dma_start(ww_t[:, :, m, :], wwT_v[:, :, m, :].bitcast(F32R))

            def body():
                for _ in range(repeat):
                    _pass()

            def _pass():
                for pr in range(NPAIR):
                    b0 = pr * PAIR
                    et = xpool.tile([128, PAIR, KT, NN], F32R, tag="et")
                    ht = xpool.tile([128, PAIR, KT, NN], F32R, tag="ht")
                    for b in range(PAIR):
                        nc.sync.dma_start(et[:, b], e_v[:, :, b0 + b, :].bitcast(F32R))
                        nc.sync.dma_start(ht[:, b], h_v[:, :, b0 + b, :].bitcast(F32R))
                    for m in range(MT):
                        ps = pspool.tile([128, NCOL], F32)
                        for k in range(KT):
                            nc.tensor.matmul(
                                ps[:], we_t[:, k, m, :], et[:, :, k, :],
                                start=(k == 0), stop=False)
                        for k in range(KT):
                            nc.tensor.matmul(
                                ps[:], ww_t[:, k, m, :], ht[:, :, k, :],
                                start=False, stop=(k == KT - 1))
                        res = opool.tile([128, PAIR, NN], F32)
                        nc.scalar.activation(
                            res[:].rearrange("p b n -> p (b n)"), ps[:],
                            mybir.ActivationFunctionType.Identity,
                            bias=bias_t[:, m:m + 1], scale=1.0)
                        nc.sync.dma_start(out_v[:, m, b0:b0 + PAIR, :], res[:])

            if loop_repeat > 1:
                with tc.For_i(0, loop_repeat, 1,
                              hint_engines=(mybir.EngineType.PE,)):
                    body()
            else:
                body()
    nc.compile()
    return nc


def _prep_in_maps(h_w, e_vw, We, be, Ww, bw):
    e_vw = np.asarray(e_vw, dtype=np.float32)
    h_w = np.asarray(h_w, dtype=np.float32)
    weT = np.ascontiguousarray(np.asarray(We, dtype=np.float32).T)
    wwT = np.ascontiguousarray(np.asarray(Ww, dtype=np.float32).T)
    bias = (np.asarray(be, dtype=np.float32)
            + np.asarray(bw, dtype=np.float32)).astype(np.float32)
    return [
        {"e": np.ascontiguousarray(e_vw[c * BPC:(c + 1) * BPC]),
         "h": np.ascontiguousarray(h_w[c * BPC:(c + 1) * BPC]),
         "weT": weT, "wwT": wwT, "bias": bias}
        for c in range(NCORES)
    ]


def kernel(h_v, h_w, e_vw, We, be, Ww, bw):
    nc = build()
    in_maps = _prep_in_maps(h_w, e_vw, We, be, Ww, bw)
    r = run_bass_kernel_spmd(nc, in_maps, core_ids=list(range(NCORES)))
    return np.concatenate(
        [r.results[c]["out"] for c in range(NCORES)], axis=0)
